# revision 43
# baseline (speedup 1.0000x reference)
"""Trainium2 Bass kernel for nn_NoiseConditionedMoE.

Strategy
--------
Routing depends only on noise_clock_emb ([B=8, 256]) -- it is computed on the
host in numpy (exactly mirroring the jax reference).  Only TOP_K=2 of E=8
experts have nonzero mixing coefficients per batch row, and the reference's
dense coeff-weighted combine makes the other 6 experts' contributions exactly
zero.  So each of the 8 NeuronCores handles one batch row and computes only
that row's two selected experts:

    per core b:
      hT[e]  = (W_in[sel_e].T @ x_b.T)            (matmul1, psum [h',s])
      gate   = silu(hT[H:2H] + b_gate)             (ACT)
      hid[e] = (hT[0:H]*w_e + b_val*w_e) * gate    (DVE, coeff folded in)
      outT  += W_out[sel_e][h,d].T @ hid[e]        (matmul2, psum [d,s])
      out    = outT + coeff-weighted fc_out bias   (ACT bias along d)

All matmul operands are bf16 (full PE rate); accumulation is fp32 in PSUM.
The host slices/reformats per-core weights, runs the SPMD kernel on cores
0..7, and transposes/gathers the result.
"""

from contextlib import ExitStack

import ml_dtypes
import numpy as np

import concourse.bass as bass
import concourse.tile as tile
from concourse import mybir
from concourse.bass_utils import run_bass_kernel_spmd

# ---------------------------------------------------------------- constants
B, S, D = 8, 1024, 1024
H = 2048
E = 8
TOP_K = 2
TEMP = 1.0
P = 128

F32 = mybir.dt.float32
BF16 = mybir.dt.bfloat16
NP_BF16 = ml_dtypes.bfloat16

DS = D // P        # 8   d subtiles
GT = H // P        # 16  h' tiles per half (value / gate)
HS = H // P        # 16  h subtiles for matmul2
ST = S // P        # 8   s tiles of 128
SH = S // 512      # 2   s halves of 512
DT2 = D // 512     # 2   d halves of 512 (matmul2 rhs free dim)


class _TC(tile.TileContext):
    """TileContext whose tail drain splits sem waits one-per-nop (this
    container's walrus rejects instructions with several sync waits)."""

    def _drain_and_barrier(self, tick_clock, wait_clock):
        from concourse.tile import ScopedClock

        nc = self.nc
        probe = nc.sync.nop()
        wait_clock.add_sem_waits(probe.ins, ScopedClock({None: tick_clock.global_clock}))
        si = probe.ins.sync_info
        waits = list(si.on_wait or []) if si else []
        if si and len(waits) > 1:
            probe.ins.sync_info = mybir.SyncInfo(
                on_wait=waits[:1], on_update=list(si.on_update or [])
            )
            for w in waits[1:]:
                n2 = nc.sync.nop()
                n2.ins.sync_info = mybir.SyncInfo(on_wait=[w], on_update=[])
        nc.sync.drain()
        nc.all_engine_barrier()
        assert self.sems is not None
        popped = nc._tile_sem_poison_stack.pop()
        assert popped is self._sem_poison
        nc.clear_and_free_semaphores(list(self.sems.allocated().values()))
        nc.all_engine_barrier()


_WAIT_BUDGET_DEFAULT = 1   # this walrus rejects multi-wait sync on most opcodes
_WAIT_BUDGET = {"InstTensorCopy": 2, "InstMatmult": 2}


def _split_waits(nc):
    """Post-pass: hoist excess sem waits onto same-engine NoOps inserted
    right before the instruction (the container's walrus has small
    per-opcode sync-wait limits)."""
    for f in nc.m.functions:
        for blk in f.blocks:
            insts = list(blk.instructions)
            out = []
            changed = False
            for inst in insts:
                si = inst.sync_info
                waits = list(si.on_wait) if si and si.on_wait else []
                budget = _WAIT_BUDGET.get(type(inst).__name__, _WAIT_BUDGET_DEFAULT)
                if len(waits) > budget and inst.engine != mybir.EngineType.Unassigned:
                    extra = waits[: len(waits) - budget]
                    keep = waits[len(waits) - budget :]
                    for i, w in enumerate(extra):
                        nop = mybir.InstNoOp(name=f"{inst.name}-ws{i}", ins=[], outs=[])
                        nop.engine = inst.engine
                        nop.sync_info = mybir.SyncInfo(on_wait=[w], on_update=[])
                        out.append(nop)
                    inst.sync_info = mybir.SyncInfo(
                        on_wait=keep, on_update=list(si.on_update or [])
                    )
                    changed = True
                out.append(inst)
            if changed:
                blk.instructions = out
    return nc


def _build_nc():
    nc = bass.Bass("TRN2", target_bir_lowering=False, debug=False, num_devices=8)

    # Per-core DRAM inputs (all host-prepared layouts).
    xt = nc.dram_tensor("xt", [DS, P, S], BF16, kind="ExternalInput").ap()
    w_in = nc.dram_tensor("w_in", [2, GT, 2, P, DS, P], BF16, kind="ExternalInput").ap()
    w_out = nc.dram_tensor("w_out", [2, HS, P, D], BF16, kind="ExternalInput").ap()
    bg = nc.dram_tensor("bg", [P, 2, GT], F32, kind="ExternalInput").ap()
    bv = nc.dram_tensor("bv", [P, 2, GT], F32, kind="ExternalInput").ap()
    wv = nc.dram_tensor("wv", [P, 2], F32, kind="ExternalInput").ap()
    br = nc.dram_tensor("br", [P, DS], F32, kind="ExternalInput").ap()
    outT = nc.dram_tensor("outT", [DS, P, S], F32, kind="ExternalOutput").ap()

    with _TC(nc) as tc:
        with ExitStack() as ctx:
            const = ctx.enter_context(tc.tile_pool(name="const", bufs=1))
            hidp = ctx.enter_context(tc.tile_pool(name="hid", bufs=1))
            winp = ctx.enter_context(tc.tile_pool(name="win", bufs=8))
            wout = ctx.enter_context(tc.tile_pool(name="wout", bufs=8))
            gatep = ctx.enter_context(tc.tile_pool(name="gate", bufs=4))
            outp = ctx.enter_context(tc.tile_pool(name="outp", bufs=6))
            # One PSUM pool, 8 bank-tags of 1 buf each (8 banks total).
            # Phase 1 rotates ps_g/ps_v through all 8; phase 2 keeps 8 live
            # accumulators (4 dl x 2 sh).
            psum = ctx.enter_context(tc.tile_pool(name="ps", bufs=1, space="PSUM"))

            def psum_tile(bank):
                return psum.tile(
                    [P, 512], F32, name=f"bank{bank}", tag=f"bank{bank}"
                )

            # --- resident tiles -------------------------------------------
            # DMA issue is spread across three engines so the start-up
            # transfers land in parallel: sync feeds wi, gpsimd the sh=0
            # half of x^T (needed by the first psum groups), scalar the rest.
            xt_sb = const.tile([P, DS, S], BF16)          # 16KB/part
            # sh=0 slices via gpsimd (8 SWDGE queues in parallel, first psum
            # groups need them), sh=1 via scalar; sync is reserved for wi.
            for ds in range(DS):
                nc.gpsimd.dma_start(xt_sb[:, ds, bass.ts(0, 512)], xt[ds, :, bass.ts(0, 512)])
            for ds in range(DS):
                nc.scalar.dma_start(xt_sb[:, ds, bass.ts(1, 512)], xt[ds, :, bass.ts(1, 512)])
            bg_sb = const.tile([P, 2, GT], F32)
            nc.scalar.dma_start(bg_sb[:], bg[:])
            bv_sb = const.tile([P, 2, GT], F32)
            nc.scalar.dma_start(bv_sb[:], bv[:])
            wv_sb = const.tile([P, 2], F32)
            nc.scalar.dma_start(wv_sb[:], wv[:])
            br_sb = const.tile([P, DS], F32)
            nc.scalar.dma_start(br_sb[:], br[:])

            hid_sb = [
                hidp.tile([P, HS, S], BF16, name=f"hid{e}", tag=f"hid{e}")
                for e in range(2)
            ]

            # --- phase 1: hT = W_in.T @ xT; hid = (v*w+bv*w)*silu(g+bg) ---
            it1 = 0
            for e in range(2):
                for t in range(GT):
                    # gate first: the first psum group of each t needs it
                    wi_g = winp.tile([P, DS, P], BF16, tag="wi_g")
                    nc.sync.dma_start(wi_g[:], w_in[e, t, 1])
                    wi_v = winp.tile([P, DS, P], BF16, tag="wi_v")
                    nc.sync.dma_start(wi_v[:], w_in[e, t, 0])
                    for sh in range(SH):
                        ps_g = psum_tile((2 * it1) % 8)
                        for ds in range(DS):
                            nc.tensor.matmul(
                                ps_g[:],
                                wi_g[:, ds, :],
                                xt_sb[:, ds, bass.ts(sh, 512)],
                                start=(ds == 0),
                                stop=(ds == DS - 1),
                            )
                        ps_v = psum_tile((2 * it1 + 1) % 8)
                        it1 += 1
                        for ds in range(DS):
                            nc.tensor.matmul(
                                ps_v[:],
                                wi_v[:, ds, :],
                                xt_sb[:, ds, bass.ts(sh, 512)],
                                start=(ds == 0),
                                stop=(ds == DS - 1),
                            )
                        gate_t = gatep.tile([P, 512], F32, tag="gate")
                        nc.scalar.activation(
                            gate_t[:],
                            ps_g[:],
                            mybir.ActivationFunctionType.Silu,
                            bias=bg_sb[:, e, t : t + 1],
                        )
                        hslice = hid_sb[e][:, t, bass.ts(sh, 512)]
                        nc.vector.tensor_scalar(
                            hslice,
                            ps_v[:],
                            wv_sb[:, e : e + 1],
                            bv_sb[:, e, t : t + 1],
                            mybir.AluOpType.mult,
                            mybir.AluOpType.add,
                        )
                        nc.vector.tensor_tensor(
                            hslice, hslice, gate_t[:], mybir.AluOpType.mult
                        )

            # --- phase 2: outT[d,s] += W_out[h,d].T @ hid[h,s] ------------
            # d processed in 2 groups of 4x128; per group 8 live psum tiles
            # (4 dl x 2 sh) accumulate over both experts' 16 h-subtiles.
            for dg in range(2):
                ps_o = [
                    [psum_tile(dl * 2 + sh) for sh in range(SH)] for dl in range(4)
                ]
                for e in range(2):
                    for hs in range(HS):
                        wo = wout.tile([P, 512], BF16, tag="wo")
                        nc.gpsimd.dma_start(wo[:], w_out[e, hs, :, bass.ts(dg, 512)])
                        for dl in range(4):
                            for sh in range(SH):
                                nc.tensor.matmul(
                                    ps_o[dl][sh][:],
                                    wo[:, bass.ts(dl, P)],
                                    hid_sb[e][:, hs, bass.ts(sh, 512)],
                                    start=(e == 0 and hs == 0),
                                    stop=(e == 1 and hs == HS - 1),
                                )
                # drain accumulators through ACT and DVE in parallel,
                # adding the coeff-weighted fc_out bias (per-partition = d).
                for dl in range(4):
                    dt = dg * 4 + dl
                    for sh in range(SH):
                        ot = outp.tile([P, 512], F32, tag="ot")
                        if dl % 2 == 0:
                            nc.scalar.activation(
                                ot[:],
                                ps_o[dl][sh][:],
                                mybir.ActivationFunctionType.Identity,
                                bias=br_sb[:, dt : dt + 1],
                            )
                        else:
                            nc.vector.tensor_scalar_add(
                                ot[:], ps_o[dl][sh][:], br_sb[:, dt : dt + 1]
                            )
                        nc.sync.dma_start(outT[dt, :, bass.ts(sh, 512)], ot[:])
    return _split_waits(nc)


_NC_CACHE = {}


def _get_nc():
    if "nc" not in _NC_CACHE:
        _NC_CACHE["nc"] = _build_nc()
    return _NC_CACHE["nc"]


def kernel(x, noise_clock_emb, route_weight, fc_in_w, fc_in_b, fc_out_w, fc_out_b):
    x = np.asarray(x)
    noise_clock_emb = np.asarray(noise_clock_emb, dtype=np.float32)
    route_weight = np.asarray(route_weight, dtype=np.float32)
    fc_in_w = np.asarray(fc_in_w)
    fc_in_b = np.asarray(fc_in_b, dtype=np.float32)
    fc_out_w = np.asarray(fc_out_w)
    fc_out_b = np.asarray(fc_out_b, dtype=np.float32)

    # ---- host router (mirrors the jax reference in fp32) ----------------
    logits = (noise_clock_emb @ route_weight) / TEMP                  # [B, E]
    mx = logits.max(axis=-1, keepdims=True)
    ex = np.exp(logits - mx)
    probs = (ex / ex.sum(axis=-1, keepdims=True)).astype(np.float32)
    topk_indices = np.argsort(-probs, axis=-1, kind="stable")[:, :TOP_K].astype(np.int32)
    topk_weights = np.take_along_axis(probs, topk_indices, axis=-1)
    topk_weights = (
        topk_weights / np.clip(topk_weights.sum(axis=-1, keepdims=True), 1e-8, None)
    ).astype(np.float32)

    # ---- per-core input prep -------------------------------------------
    fc_in_w_bf = fc_in_w.astype(NP_BF16)
    fc_out_w_bf = fc_out_w.astype(NP_BF16)

    in_maps = []
    for b in range(B):
        sel = topk_indices[b]
        w = topk_weights[b]
        # x^T tiles: [DS, P, S]
        xt = np.ascontiguousarray(x[b].T.astype(NP_BF16)).reshape(DS, P, S)
        # W_in tiles: [2, GT, 2(v/g), P, DS, P];
        # w_in[e, t, vg, p, ds, hp] = W[ds*P+p, vg*H + t*P + hp]
        wi = (
            fc_in_w_bf[sel]
            .reshape(2, DS, P, 2, GT, P)
            .transpose(0, 4, 3, 2, 1, 5)
        )
        wi = np.ascontiguousarray(wi)
        # W_out tiles: [2, HS, P, D]
        wo = np.ascontiguousarray(fc_out_w_bf[sel].reshape(2, HS, P, D))
        # biases
        b_in = fc_in_b[sel]                                # [2, 2H]
        bg = np.ascontiguousarray(
            np.broadcast_to(
                b_in[:, H:].reshape(2, GT, P).transpose(2, 0, 1), (P, 2, GT)
            )
        )
        # value bias scaled by coeff
        bvs = b_in[:, :H] * w[:, None]                     # [2, H]
        bv = np.ascontiguousarray(bvs.reshape(2, GT, P).transpose(2, 0, 1))
        wv = np.ascontiguousarray(np.broadcast_to(w[None, :], (P, 2))).astype(np.float32)
        br_vec = (w[:, None] * fc_out_b[sel]).sum(axis=0)  # [D]
        br = np.ascontiguousarray(br_vec.reshape(DS, P).T).astype(np.float32)
        in_maps.append(
            {
                "xt": xt,
                "w_in": wi,
                "w_out": wo,
                "bg": np.ascontiguousarray(bg, dtype=np.float32),
                "bv": np.ascontiguousarray(bv, dtype=np.float32),
                "wv": wv,
                "br": br,
            }
        )

    nc = _get_nc()
    _NC_CACHE["last_in_maps"] = in_maps
    res = run_bass_kernel_spmd(nc, in_maps, core_ids=list(range(B)))

    mixed = np.empty((B, S, D), dtype=np.float32)
    for b in range(B):
        oT = res.results[b]["outT"].reshape(D, S)
        mixed[b] = oT.T

    return mixed, logits, probs, topk_indices, topk_weights


# revision 44
# speedup vs baseline: 1.0162x; 1.0162x over previous
"""Trainium2 Bass kernel for nn_NoiseConditionedMoE.

Strategy
--------
Routing depends only on noise_clock_emb ([B=8, 256]) -- it is computed on the
host in numpy (exactly mirroring the jax reference).  Only TOP_K=2 of E=8
experts have nonzero mixing coefficients per batch row, and the reference's
dense coeff-weighted combine makes the other 6 experts' contributions exactly
zero.  So each of the 8 NeuronCores handles one batch row and computes only
that row's two selected experts:

    per core b:
      hT[e]  = (W_in[sel_e].T @ x_b.T)            (matmul1, psum [h',s])
      gate   = silu(hT[H:2H] + b_gate)             (ACT)
      hid[e] = (hT[0:H]*w_e + b_val*w_e) * gate    (DVE, coeff folded in)
      outT  += W_out[sel_e][h,d].T @ hid[e]        (matmul2, psum [d,s])
      out    = outT + coeff-weighted fc_out bias   (ACT bias along d)

All matmul operands are bf16 (full PE rate); accumulation is fp32 in PSUM.
The host slices/reformats per-core weights, runs the SPMD kernel on cores
0..7, and transposes/gathers the result.
"""

from contextlib import ExitStack

import ml_dtypes
import numpy as np

import concourse.bass as bass
import concourse.tile as tile
from concourse import mybir
from concourse.bass_utils import run_bass_kernel_spmd

# ---------------------------------------------------------------- constants
B, S, D = 8, 1024, 1024
H = 2048
E = 8
TOP_K = 2
TEMP = 1.0
P = 128

F32 = mybir.dt.float32
BF16 = mybir.dt.bfloat16
NP_BF16 = ml_dtypes.bfloat16

DS = D // P        # 8   d subtiles
GT = H // P        # 16  h' tiles per half (value / gate)
HS = H // P        # 16  h subtiles for matmul2
ST = S // P        # 8   s tiles of 128
SH = S // 512      # 2   s halves of 512
DT2 = D // 512     # 2   d halves of 512 (matmul2 rhs free dim)


class _TC(tile.TileContext):
    """TileContext whose tail drain splits sem waits one-per-nop (this
    container's walrus rejects instructions with several sync waits)."""

    def _drain_and_barrier(self, tick_clock, wait_clock):
        from concourse.tile import ScopedClock

        nc = self.nc
        probe = nc.sync.nop()
        wait_clock.add_sem_waits(probe.ins, ScopedClock({None: tick_clock.global_clock}))
        si = probe.ins.sync_info
        waits = list(si.on_wait or []) if si else []
        if si and len(waits) > 1:
            probe.ins.sync_info = mybir.SyncInfo(
                on_wait=waits[:1], on_update=list(si.on_update or [])
            )
            for w in waits[1:]:
                n2 = nc.sync.nop()
                n2.ins.sync_info = mybir.SyncInfo(on_wait=[w], on_update=[])
        nc.sync.drain()
        nc.all_engine_barrier()
        assert self.sems is not None
        popped = nc._tile_sem_poison_stack.pop()
        assert popped is self._sem_poison
        nc.clear_and_free_semaphores(list(self.sems.allocated().values()))
        nc.all_engine_barrier()


_WAIT_BUDGET_DEFAULT = 1   # this walrus rejects multi-wait sync on most opcodes
_WAIT_BUDGET = {"InstTensorCopy": 2, "InstMatmult": 2}


def _split_waits(nc):
    """Post-pass: hoist excess sem waits onto same-engine NoOps inserted
    right before the instruction (the container's walrus has small
    per-opcode sync-wait limits)."""
    for f in nc.m.functions:
        for blk in f.blocks:
            insts = list(blk.instructions)
            out = []
            changed = False
            for inst in insts:
                si = inst.sync_info
                waits = list(si.on_wait) if si and si.on_wait else []
                budget = _WAIT_BUDGET.get(type(inst).__name__, _WAIT_BUDGET_DEFAULT)
                if len(waits) > budget and inst.engine != mybir.EngineType.Unassigned:
                    extra = waits[: len(waits) - budget]
                    keep = waits[len(waits) - budget :]
                    for i, w in enumerate(extra):
                        nop = mybir.InstNoOp(name=f"{inst.name}-ws{i}", ins=[], outs=[])
                        nop.engine = inst.engine
                        nop.sync_info = mybir.SyncInfo(on_wait=[w], on_update=[])
                        out.append(nop)
                    inst.sync_info = mybir.SyncInfo(
                        on_wait=keep, on_update=list(si.on_update or [])
                    )
                    changed = True
                out.append(inst)
            if changed:
                blk.instructions = out
    return nc


def _build_nc():
    nc = bass.Bass("TRN2", target_bir_lowering=False, debug=False, num_devices=8)

    # Per-core DRAM inputs (all host-prepared layouts).
    xt = nc.dram_tensor("xt", [DS, P, S], BF16, kind="ExternalInput").ap()
    w_in = nc.dram_tensor("w_in", [2, GT, 2, P, DS, P], BF16, kind="ExternalInput").ap()
    w_out = nc.dram_tensor("w_out", [2, HS, P, D], BF16, kind="ExternalInput").ap()
    bg = nc.dram_tensor("bg", [P, 2, GT], F32, kind="ExternalInput").ap()
    bv = nc.dram_tensor("bv", [P, 2, GT], F32, kind="ExternalInput").ap()
    wv = nc.dram_tensor("wv", [P, 2], F32, kind="ExternalInput").ap()
    br = nc.dram_tensor("br", [P, DS], F32, kind="ExternalInput").ap()
    outT = nc.dram_tensor("outT", [DS, P, S], F32, kind="ExternalOutput").ap()

    with _TC(nc) as tc:
        with ExitStack() as ctx:
            const = ctx.enter_context(tc.tile_pool(name="const", bufs=1))
            hidp = ctx.enter_context(tc.tile_pool(name="hid", bufs=1))
            winp = ctx.enter_context(tc.tile_pool(name="win", bufs=8))
            wout = ctx.enter_context(tc.tile_pool(name="wout", bufs=8))
            gatep = ctx.enter_context(tc.tile_pool(name="gate", bufs=4))
            outp = ctx.enter_context(tc.tile_pool(name="outp", bufs=6))
            # One PSUM pool, 8 bank-tags of 1 buf each (8 banks total).
            # Phase 1 rotates ps_g/ps_v through all 8; phase 2 keeps 8 live
            # accumulators (4 dl x 2 sh).
            psum = ctx.enter_context(tc.tile_pool(name="ps", bufs=1, space="PSUM"))

            def psum_tile(bank):
                return psum.tile(
                    [P, 512], F32, name=f"bank{bank}", tag=f"bank{bank}"
                )

            # --- resident tiles -------------------------------------------
            # DMA issue is spread across three engines so the start-up
            # transfers land in parallel: sync feeds wi, gpsimd the sh=0
            # half of x^T (needed by the first psum groups), scalar the rest.
            xt_sb = const.tile([P, DS, S], BF16)          # 16KB/part
            # tiny consts first on scalar (ACT needs bg for the first silu),
            # then x^T: sh=0 via gpsimd (first psum groups), sh=1 split
            # gpsimd/scalar; sync is reserved for the wi stream.
            bg_sb = const.tile([P, 2, GT], F32)
            nc.scalar.dma_start(bg_sb[:], bg[:])
            bv_sb = const.tile([P, 2, GT], F32)
            nc.scalar.dma_start(bv_sb[:], bv[:])
            wv_sb = const.tile([P, 2], F32)
            nc.scalar.dma_start(wv_sb[:], wv[:])
            br_sb = const.tile([P, DS], F32)
            nc.scalar.dma_start(br_sb[:], br[:])
            for ds in range(DS):
                nc.gpsimd.dma_start(xt_sb[:, ds, bass.ts(0, 512)], xt[ds, :, bass.ts(0, 512)])
            for ds in range(DS):
                eng = nc.scalar if ds >= 4 else nc.gpsimd
                eng.dma_start(xt_sb[:, ds, bass.ts(1, 512)], xt[ds, :, bass.ts(1, 512)])

            hid_sb = [
                hidp.tile([P, HS, S], BF16, name=f"hid{e}", tag=f"hid{e}")
                for e in range(2)
            ]

            # --- phase 1: hT = W_in.T @ xT; hid = (v*w+bv*w)*silu(g+bg) ---
            it1 = 0
            for e in range(2):
                for t in range(GT):
                    # gate first: the first psum group of each t needs it
                    wi_g = winp.tile([P, DS, P], BF16, tag="wi_g")
                    nc.sync.dma_start(wi_g[:], w_in[e, t, 1])
                    wi_v = winp.tile([P, DS, P], BF16, tag="wi_v")
                    nc.sync.dma_start(wi_v[:], w_in[e, t, 0])
                    for sh in range(SH):
                        ps_g = psum_tile((2 * it1) % 8)
                        for ds in range(DS):
                            nc.tensor.matmul(
                                ps_g[:],
                                wi_g[:, ds, :],
                                xt_sb[:, ds, bass.ts(sh, 512)],
                                start=(ds == 0),
                                stop=(ds == DS - 1),
                            )
                        ps_v = psum_tile((2 * it1 + 1) % 8)
                        it1 += 1
                        for ds in range(DS):
                            nc.tensor.matmul(
                                ps_v[:],
                                wi_v[:, ds, :],
                                xt_sb[:, ds, bass.ts(sh, 512)],
                                start=(ds == 0),
                                stop=(ds == DS - 1),
                            )
                        gate_t = gatep.tile([P, 512], F32, tag="gate")
                        nc.scalar.activation(
                            gate_t[:],
                            ps_g[:],
                            mybir.ActivationFunctionType.Silu,
                            bias=bg_sb[:, e, t : t + 1],
                        )
                        hslice = hid_sb[e][:, t, bass.ts(sh, 512)]
                        nc.vector.tensor_scalar(
                            hslice,
                            ps_v[:],
                            wv_sb[:, e : e + 1],
                            bv_sb[:, e, t : t + 1],
                            mybir.AluOpType.mult,
                            mybir.AluOpType.add,
                        )
                        nc.vector.tensor_tensor(
                            hslice, hslice, gate_t[:], mybir.AluOpType.mult
                        )

            # --- phase 2: outT[d,s] += W_out[h,d].T @ hid[h,s] ------------
            # d processed in 2 groups of 4x128; per group 8 live psum tiles
            # (4 dl x 2 sh) accumulate over both experts' 16 h-subtiles.
            for dg in range(2):
                ps_o = [
                    [psum_tile(dl * 2 + sh) for sh in range(SH)] for dl in range(4)
                ]
                for e in range(2):
                    for hs in range(HS):
                        wo = wout.tile([P, 512], BF16, tag="wo")
                        nc.gpsimd.dma_start(wo[:], w_out[e, hs, :, bass.ts(dg, 512)])
                        for dl in range(4):
                            for sh in range(SH):
                                nc.tensor.matmul(
                                    ps_o[dl][sh][:],
                                    wo[:, bass.ts(dl, P)],
                                    hid_sb[e][:, hs, bass.ts(sh, 512)],
                                    start=(e == 0 and hs == 0),
                                    stop=(e == 1 and hs == HS - 1),
                                )
                # drain accumulators through ACT and DVE in parallel,
                # adding the coeff-weighted fc_out bias (per-partition = d).
                for dl in range(4):
                    dt = dg * 4 + dl
                    for sh in range(SH):
                        ot = outp.tile([P, 512], F32, tag="ot")
                        if dl % 2 == 0:
                            nc.scalar.activation(
                                ot[:],
                                ps_o[dl][sh][:],
                                mybir.ActivationFunctionType.Identity,
                                bias=br_sb[:, dt : dt + 1],
                            )
                        else:
                            nc.vector.tensor_scalar_add(
                                ot[:], ps_o[dl][sh][:], br_sb[:, dt : dt + 1]
                            )
                        nc.sync.dma_start(outT[dt, :, bass.ts(sh, 512)], ot[:])
    return _split_waits(nc)


_NC_CACHE = {}


def _get_nc():
    if "nc" not in _NC_CACHE:
        _NC_CACHE["nc"] = _build_nc()
    return _NC_CACHE["nc"]


def kernel(x, noise_clock_emb, route_weight, fc_in_w, fc_in_b, fc_out_w, fc_out_b):
    x = np.asarray(x)
    noise_clock_emb = np.asarray(noise_clock_emb, dtype=np.float32)
    route_weight = np.asarray(route_weight, dtype=np.float32)
    fc_in_w = np.asarray(fc_in_w)
    fc_in_b = np.asarray(fc_in_b, dtype=np.float32)
    fc_out_w = np.asarray(fc_out_w)
    fc_out_b = np.asarray(fc_out_b, dtype=np.float32)

    # ---- host router (mirrors the jax reference in fp32) ----------------
    logits = (noise_clock_emb @ route_weight) / TEMP                  # [B, E]
    mx = logits.max(axis=-1, keepdims=True)
    ex = np.exp(logits - mx)
    probs = (ex / ex.sum(axis=-1, keepdims=True)).astype(np.float32)
    topk_indices = np.argsort(-probs, axis=-1, kind="stable")[:, :TOP_K].astype(np.int32)
    topk_weights = np.take_along_axis(probs, topk_indices, axis=-1)
    topk_weights = (
        topk_weights / np.clip(topk_weights.sum(axis=-1, keepdims=True), 1e-8, None)
    ).astype(np.float32)

    # ---- per-core input prep -------------------------------------------
    fc_in_w_bf = fc_in_w.astype(NP_BF16)
    fc_out_w_bf = fc_out_w.astype(NP_BF16)

    in_maps = []
    for b in range(B):
        sel = topk_indices[b]
        w = topk_weights[b]
        # x^T tiles: [DS, P, S]
        xt = np.ascontiguousarray(x[b].T.astype(NP_BF16)).reshape(DS, P, S)
        # W_in tiles: [2, GT, 2(v/g), P, DS, P];
        # w_in[e, t, vg, p, ds, hp] = W[ds*P+p, vg*H + t*P + hp]
        wi = (
            fc_in_w_bf[sel]
            .reshape(2, DS, P, 2, GT, P)
            .transpose(0, 4, 3, 2, 1, 5)
        )
        wi = np.ascontiguousarray(wi)
        # W_out tiles: [2, HS, P, D]
        wo = np.ascontiguousarray(fc_out_w_bf[sel].reshape(2, HS, P, D))
        # biases
        b_in = fc_in_b[sel]                                # [2, 2H]
        bg = np.ascontiguousarray(
            np.broadcast_to(
                b_in[:, H:].reshape(2, GT, P).transpose(2, 0, 1), (P, 2, GT)
            )
        )
        # value bias scaled by coeff
        bvs = b_in[:, :H] * w[:, None]                     # [2, H]
        bv = np.ascontiguousarray(bvs.reshape(2, GT, P).transpose(2, 0, 1))
        wv = np.ascontiguousarray(np.broadcast_to(w[None, :], (P, 2))).astype(np.float32)
        br_vec = (w[:, None] * fc_out_b[sel]).sum(axis=0)  # [D]
        br = np.ascontiguousarray(br_vec.reshape(DS, P).T).astype(np.float32)
        in_maps.append(
            {
                "xt": xt,
                "w_in": wi,
                "w_out": wo,
                "bg": np.ascontiguousarray(bg, dtype=np.float32),
                "bv": np.ascontiguousarray(bv, dtype=np.float32),
                "wv": wv,
                "br": br,
            }
        )

    nc = _get_nc()
    _NC_CACHE["last_in_maps"] = in_maps
    res = run_bass_kernel_spmd(nc, in_maps, core_ids=list(range(B)))

    mixed = np.empty((B, S, D), dtype=np.float32)
    for b in range(B):
        oT = res.results[b]["outT"].reshape(D, S)
        mixed[b] = oT.T

    return mixed, logits, probs, topk_indices, topk_weights
